# revision 6
# baseline (speedup 1.0000x reference)
"""Trainium2 Bass kernel for nn_DGCRM (GRU-style recurrent graph model).

Math per step t (per batch b):
  inp  = [x_t, state]                                  # [N, 66]
  zr   = sigmoid(inp @ Wg_al + bg_al + s0g * agg_g)    # agg_g = sum_n relu(inp @ Wg + bg)
  z, r = zr[:, :64], zr[:, 64:]
  cand = [x_t, z * state]
  hc   = tanh(cand @ Wu_al + bu_al + s0u * agg_u)      # agg_u = sum_n relu(cand @ Wu + bu)
  state = r * state + (1 - r) * hc

The rank-1 "diffusion" term s[n] * agg[c] collapses to a per-channel bias
because setup_inputs() uses uniform node/add weights (s[n] == s0) and
aff_w == 1, aff_b == 0 (verified at runtime; general fallback otherwise).

Device layout is channel-major: state lives in SBUF as [67, N] per batch
(rows 0:64 state, 64:66 x_t, row 66 = const 1.0 so matmul K-rows carry the
x-part and the bias). All matmuls are K<=67 with weights stationary; node
dim streams through PE in <=512-column slices. Sharding: data-parallel over
batch, 2 batches per core, no collectives. Output is written channel-major
[b, t, 64, N] and transposed to [b, t, N, 64] on the host during unshard.
"""

import os
import numpy as np

N = 8600
DIN = 2
H = 64
B = 16
T = 12
NCORES = 8
BPC = B // NCORES  # batches per core


# ---------------------------------------------------------------------------
# device program
# ---------------------------------------------------------------------------

def _build_program(n, t_steps, chunk, s0g, s0u):
    import concourse.bacc as bacc
    import concourse.tile as tile  # noqa: F401
    from concourse import mybir
    import concourse.tile as tile_mod

    f32 = mybir.dt.float32
    AF = mybir.ActivationFunctionType

    # node-dim chunks (elementwise granularity) and <=512 matmul slices
    chunks = []
    c0 = 0
    while c0 < n:
        chunks.append((c0, min(chunk, n - c0)))
        c0 += chunk
    nch = len(chunks)

    def mm_slices(cw):
        out, s = [], 0
        while s < cw:
            out.append((s, min(512, cw - s)))
            s += 512
        return out

    nc = bacc.Bacc("TRN2", target_bir_lowering=False, debug=False)

    xt_d = nc.dram_tensor("xt", [BPC, t_steps, DIN, n], f32, kind="ExternalInput")
    st0_d = nc.dram_tensor("st0", [BPC, H, n], f32, kind="ExternalInput")
    wgw_d = nc.dram_tensor("wgw", [67, 128], f32, kind="ExternalInput")
    wga_d = nc.dram_tensor("wga", [67, 128], f32, kind="ExternalInput")
    wuz_d = nc.dram_tensor("wuz", [64, 64], f32, kind="ExternalInput")
    wux_d = nc.dram_tensor("wux", [3, 64], f32, kind="ExternalInput")
    wuaz_d = nc.dram_tensor("wuaz", [64, 64], f32, kind="ExternalInput")
    wuax_d = nc.dram_tensor("wuax", [3, 64], f32, kind="ExternalInput")
    out_d = nc.dram_tensor("out_cm", [BPC, t_steps, H, n], f32, kind="ExternalOutput")

    with tile_mod.TileContext(nc) as tc:
        with (
            tc.tile_pool(name="persist", bufs=1) as persist,
            tc.tile_pool(name="psum", bufs=2, space="PSUM") as psum,
            tc.tile_pool(name="cand", bufs=2) as cpool,
            tc.tile_pool(name="scr", bufs=1) as scr,
            tc.tile_pool(name="hcp", bufs=2) as hcp,
            tc.tile_pool(name="smalls", bufs=2) as smalls,
        ):
            S = []
            ZR = []
            for b in range(BPC):
                sb = persist.tile([67, n], f32, name=f"S{b}")
                zrb = persist.tile([128, n], f32, name=f"ZR{b}")
                S.append(sb)
                ZR.append(zrb)
            wgw = persist.tile([67, 128], f32, name="wgw_sb")
            wga = persist.tile([67, 128], f32, name="wga_sb")
            wuz = persist.tile([64, 64], f32, name="wuz_sb")
            # x-part weights live at base partition 64 to match the rhs
            # (S rows 64:67) — matmul requires equal base partitions.
            wuxh = persist.tile([67, 64], f32, name="wux_sb")
            wuaz = persist.tile([64, 64], f32, name="wuaz_sb")
            wuaxh = persist.tile([67, 64], f32, name="wuax_sb")
            wux = wuxh[64:67, :]
            wuax = wuaxh[64:67, :]

            nc.sync.dma_start(wgw[:], wgw_d[:])
            nc.sync.dma_start(wga[:], wga_d[:])
            nc.sync.dma_start(wuz[:], wuz_d[:])
            nc.sync.dma_start(wux, wux_d[:])
            nc.sync.dma_start(wuaz[:], wuaz_d[:])
            nc.sync.dma_start(wuax, wuax_d[:])

            rscr = scr.tile([128, chunk], f32, name="rscr")

            for b in range(BPC):
                nc.sync.dma_start(S[b][0:64, :], st0_d[b])
                # row 66 must be 1.0 (bias row); rows 64:66 are overwritten
                # by the x DMA each step. Engines need 32-aligned start
                # partitions, so memset all of 64:67.
                nc.vector.memset(S[b][64:67, :], 1.0)

            for t in range(t_steps):
                for b in range(BPC):
                    Sb, ZRb = S[b], ZR[b]
                    # x_t into rows 64:66 (row 66 stays 1.0)
                    nc.sync.dma_start(Sb[64:66, :], xt_d[b, t])

                    # ---- pass 1: gate_w -> agg_g ------------------------------
                    partg = smalls.tile([128, nch], f32, tag="partg",
                                        name=f"partg_{t}_{b}")
                    for ci, (cs, cw) in enumerate(chunks):
                        ps = psum.tile([128, chunk], f32, tag="ps",
                                       name=f"ps_gw_{t}_{b}_{ci}")
                        for ss, sw in mm_slices(cw):
                            nc.tensor.matmul(
                                ps[:, ss:ss + sw], wgw[:],
                                Sb[:, cs + ss:cs + ss + sw],
                                start=True, stop=True)
                        nc.scalar.activation(
                            rscr[:, :cw], ps[:, :cw], AF.Relu,
                            accum_out=partg[:, ci:ci + 1])
                    aggg = smalls.tile([128, 1], f32, tag="aggg",
                                       name=f"aggg_{t}_{b}")
                    nc.vector.reduce_sum(aggg, partg[:, 0:nch],
                                         axis=mybir.AxisListType.X)
                    biasg = smalls.tile([128, 1], f32, tag="biasg",
                                        name=f"biasg_{t}_{b}")
                    nc.vector.tensor_scalar_mul(biasg, aggg, float(s0g))

                    # ---- pass 2: gate_align -> sigmoid -> ZR ------------------
                    for ci, (cs, cw) in enumerate(chunks):
                        ps = psum.tile([128, chunk], f32, tag="ps",
                                       name=f"ps_ga_{t}_{b}_{ci}")
                        for ss, sw in mm_slices(cw):
                            nc.tensor.matmul(
                                ps[:, ss:ss + sw], wga[:],
                                Sb[:, cs + ss:cs + ss + sw],
                                start=True, stop=True)
                        nc.scalar.activation(
                            ZRb[:, cs:cs + cw], ps[:, :cw], AF.Sigmoid,
                            bias=biasg[:, 0:1])

                    # ---- pass 3: cand (gpsimd) + upd_w -> agg_u ---------------
                    partu = smalls.tile([64, nch], f32, tag="partu",
                                        name=f"partu_{t}_{b}")
                    for ci, (cs, cw) in enumerate(chunks):
                        cc = cpool.tile([64, chunk], f32, tag="cc",
                                        name=f"cc3_{t}_{b}_{ci}")
                        nc.gpsimd.tensor_mul(cc[:, :cw], ZRb[0:64, cs:cs + cw],
                                             Sb[0:64, cs:cs + cw])
                        ps = psum.tile([64, chunk], f32, tag="ps",
                                       name=f"ps_uw_{t}_{b}_{ci}")
                        for ss, sw in mm_slices(cw):
                            nc.tensor.matmul(
                                ps[:, ss:ss + sw], wux,
                                Sb[64:67, cs + ss:cs + ss + sw],
                                start=True, stop=False)
                            nc.tensor.matmul(
                                ps[:, ss:ss + sw], wuz[:],
                                cc[:, ss:ss + sw],
                                start=False, stop=True)
                        nc.scalar.activation(
                            rscr[0:64, :cw], ps[:, :cw], AF.Relu,
                            accum_out=partu[:, ci:ci + 1])
                    aggu = smalls.tile([64, 1], f32, tag="aggu",
                                       name=f"aggu_{t}_{b}")
                    nc.vector.reduce_sum(aggu, partu[:, 0:nch],
                                         axis=mybir.AxisListType.X)
                    biasu = smalls.tile([64, 1], f32, tag="biasu",
                                        name=f"biasu_{t}_{b}")
                    nc.vector.tensor_scalar_mul(biasu, aggu, float(s0u))

                    # ---- pass 4: cand (dve) + upd_align -> tanh -> update -----
                    for ci, (cs, cw) in enumerate(chunks):
                        cc = cpool.tile([64, chunk], f32, tag="cc",
                                        name=f"cc4_{t}_{b}_{ci}")
                        nc.vector.tensor_mul(cc[:, :cw], ZRb[0:64, cs:cs + cw],
                                             Sb[0:64, cs:cs + cw])
                        ps = psum.tile([64, chunk], f32, tag="ps",
                                       name=f"ps_ua_{t}_{b}_{ci}")
                        for ss, sw in mm_slices(cw):
                            nc.tensor.matmul(
                                ps[:, ss:ss + sw], wuax,
                                Sb[64:67, cs + ss:cs + ss + sw],
                                start=True, stop=False)
                            nc.tensor.matmul(
                                ps[:, ss:ss + sw], wuaz[:],
                                cc[:, ss:ss + sw],
                                start=False, stop=True)
                        hc = hcp.tile([64, chunk], f32, tag="hc",
                                      name=f"hc_{t}_{b}_{ci}")
                        nc.scalar.activation(hc[:, :cw], ps[:, :cw], AF.Tanh,
                                             bias=biasu[:, 0:1])
                        # state = hc + r * (state - hc). r sits at partitions
                        # 64:128 while state/hc sit at 0:64; SBUF-SBUF tensor
                        # ops must share a start partition, so route d/u
                        # through the (now dead) res_u PSUM region, which is
                        # exempt from that check.
                        nc.vector.tensor_sub(ps[:, :cw],
                                             Sb[0:64, cs:cs + cw], hc[:, :cw])
                        nc.vector.tensor_mul(ps[:, :cw],
                                             ZRb[64:128, cs:cs + cw],
                                             ps[:, :cw])
                        nc.vector.tensor_add(Sb[0:64, cs:cs + cw],
                                             hc[:, :cw], ps[:, :cw])

                    nc.sync.dma_start(out_d[b, t], Sb[0:64, :])

    nc.compile()
    return nc


_PROG_CACHE = {}


def _get_program(n, t_steps, chunk, s0g, s0u):
    key = (n, t_steps, chunk, float(s0g), float(s0u))
    if key not in _PROG_CACHE:
        _PROG_CACHE[key] = _build_program(n, t_steps, chunk, s0g, s0u)
    return _PROG_CACHE[key]


# ---------------------------------------------------------------------------
# host-side packing / fallback
# ---------------------------------------------------------------------------

def _pack_weights(align_w, align_b, w, b):
    """Row order must match S rows: [state(0:64), x(64:66), ones(66)]."""
    wg = np.concatenate([w[DIN:], w[:DIN], b[None, :]], axis=0)
    wa = np.concatenate([align_w[DIN:], align_w[:DIN], align_b[None, :]], axis=0)
    return np.ascontiguousarray(wa, np.float32), np.ascontiguousarray(wg, np.float32)


def _numpy_reference(x, init_state, kw):
    """Faithful general fallback (and testing oracle)."""
    def gfs(inp, aw, ab, w, b, nw, adw, afw, afb):
        res = inp @ aw + ab
        h = np.maximum(inp @ w + b, 0.0)
        s = adw[:, 0] * nw[0, :]
        agg = h.sum(axis=1)
        diff = s[None, :, None] * agg[:, None, :]
        return res + afw * diff + afb

    state = init_state[0].astype(np.float32)
    states = []
    for t in range(x.shape[1]):
        xt = x[:, t]
        inp = np.concatenate([xt, state], axis=-1)
        zr = 1.0 / (1.0 + np.exp(-gfs(inp, kw['gate_align_w'], kw['gate_align_b'],
                                      kw['gate_w'], kw['gate_b'], kw['gate_node_w'],
                                      kw['gate_add_w'], kw['gate_aff_w'],
                                      kw['gate_aff_b'])))
        z, r = zr[..., :H], zr[..., H:]
        cand = np.concatenate([xt, z * state], axis=-1)
        hc = np.tanh(gfs(cand, kw['upd_align_w'], kw['upd_align_b'],
                         kw['upd_w'], kw['upd_b'], kw['upd_node_w'],
                         kw['upd_add_w'], kw['upd_aff_w'], kw['upd_aff_b']))
        state = r * state + (1.0 - r) * hc
        states.append(state.copy())
    out = np.stack(states, axis=1).astype(np.float32)
    return out, np.ascontiguousarray(out[:, -1])


def kernel(**inputs):
    x = np.ascontiguousarray(np.asarray(inputs['x'], np.float32))
    init_state = np.asarray(inputs['init_state'], np.float32)
    kw = {k: np.asarray(v, np.float32) for k, v in inputs.items()
          if k not in ('x', 'init_state', 'node_emb0', 'node_emb1')}

    b_, t_, n_, d_ = x.shape

    s_g = kw['gate_add_w'][:, 0] * kw['gate_node_w'][0, :]
    s_u = kw['upd_add_w'][:, 0] * kw['upd_node_w'][0, :]

    def _uniform(v):
        return np.all(v == v.flat[0])

    structured = (
        b_ == B and t_ == T and n_ == N and d_ == DIN
        and _uniform(s_g) and _uniform(s_u)
        and np.all(kw['gate_aff_w'] == 1.0) and np.all(kw['gate_aff_b'] == 0.0)
        and np.all(kw['upd_aff_w'] == 1.0) and np.all(kw['upd_aff_b'] == 0.0)
    )
    if not structured:
        return _numpy_reference(x, init_state, kw)

    s0g = float(s_g.flat[0])
    s0u = float(s_u.flat[0])

    wga, wgw = _pack_weights(kw['gate_align_w'], kw['gate_align_b'],
                             kw['gate_w'], kw['gate_b'])
    wuz = np.ascontiguousarray(kw['upd_w'][DIN:], np.float32)
    wux = np.ascontiguousarray(
        np.concatenate([kw['upd_w'][:DIN], kw['upd_b'][None, :]], axis=0), np.float32)
    wuaz = np.ascontiguousarray(kw['upd_align_w'][DIN:], np.float32)
    wuax = np.ascontiguousarray(
        np.concatenate([kw['upd_align_w'][:DIN], kw['upd_align_b'][None, :]], axis=0),
        np.float32)

    xt_all = np.ascontiguousarray(x.transpose(0, 1, 3, 2))          # [B,T,2,N]
    st0_all = np.ascontiguousarray(init_state[0].transpose(0, 2, 1))  # [B,64,N]

    chunk = 2048
    nc = _get_program(N, T, chunk, s0g, s0u)

    from concourse.bass_utils import run_bass_kernel_spmd
    in_maps = []
    for c in range(NCORES):
        sl = slice(BPC * c, BPC * (c + 1))
        in_maps.append(dict(
            xt=xt_all[sl], st0=st0_all[sl],
            wgw=wgw, wga=wga, wuz=wuz, wux=wux, wuaz=wuaz, wuax=wuax,
        ))

    trace = os.environ.get("DGCRM_TRACE", "0") == "1"
    res = run_bass_kernel_spmd(nc, in_maps, core_ids=list(range(NCORES)),
                               trace=trace)
    if trace and res.exec_time_ns is not None:
        kernel._last_exec_time_ns = res.exec_time_ns
        kernel._last_trace = res.instructions_and_trace
    out_cm = np.concatenate([r["out_cm"] for r in res.results], axis=0)  # [B,T,64,N]
    states = np.ascontiguousarray(out_cm.transpose(0, 1, 3, 2))          # [B,T,N,64]
    last = np.ascontiguousarray(states[:, -1])
    return states, last


kernel._last_exec_time_ns = None
kernel._last_trace = None
